# revision 18
# baseline (speedup 1.0000x reference)
"""RBF-kernel SVM decision function on 8 TRN2 NeuronCores.

out[i] = sum_j alphas[j] * exp(-GAMMA * ||x[i] - supports[j]||^2)

Strategy (data-parallel over x rows; supports/alphas replicated):
  exponent e_ij is produced ENTIRELY by one bf16 matmul with 68
  contraction rows:
    rows 0-63 : (x/32)^T vs s^T          -> 2*gamma*(x.s)
    row 64,65 : 1.0     vs jt hi/lo      -> ln|a_j| - gamma*|s_j|^2
    row 66,67 : c hi/lo vs 1.0           -> -gamma*|x_i|^2
  so PSUM holds e_ij directly (no ACT bias / DVE per-partition scalar).

  The N*M elementwise exp+reduce work is split between BOTH engines,
  each 2048-wide j-window handled by exactly one of them:
    ACT window: ACTIVATE(Exp, accum_out=...) in place on PSUM; the
      free-dim accumulator does the reduction for free.
    DVE window: "Schraudolph" exp — one tensor_scalar computes
      round(A*e + B) into an int16 SBUF tile; those int16 bit patterns
      ARE bf16 exp values (A = 128/ln2, B = 16256 - C).  The window's
      equal-size P and N halves are folded with one bf16
      tensor_tensor subtract (P - N), then a single 16-bit
      tensor_scalar(accum_out=...) reduces the fold (plus a tiny
      leftover reduce when the halves differ in size).
  Per-tile accumulator-column sums and the final P-N combine run on the
  otherwise idle GPSIMD engine.

  Support permutation (host-side) controls accuracy:
    w0 = largest-|alpha| positives      (always ACT: exact exp)
    w3 = largest-|alpha| negatives      (always ACT)
    w2 = [1024 smallest N | 1024 smallest P]     (always DVE)
    w1 = [medium-small N | medium-small P]       (DVE on some tiles)
  DVE windows contain only tiny-|alpha| supports (~7% of the alpha^2
  mass) AND are internally sign-balanced, so the ~1.75% rms periodic
  Schraudolph error and its global bias both wash out: simulated rel
  err ~5e-3 vs the 2e-2 gate, insensitive to the rounding mode of the
  fp32->int16 convert.  (One support — P count is odd — is parked in
  w3 with jt=-50, i.e. weight exp(-50)=0, to keep piece boundaries
  even for DVE 2x alignment.)

DVE_PER_TILE (ACT:DVE window ratio) is the main speed tuning knob.
"""

import os
import sys

for p in ("/opt/trn_rl_repo",):
    if p not in sys.path:
        sys.path.insert(0, p)

import numpy as np
import ml_dtypes

import concourse.bass as bass
import concourse.tile as tile
from concourse import bacc, mybir
from concourse.bass_utils import run_bass_kernel_spmd

N_CORES = 8
N = 16384
M = 8192
F = 64
GAMMA = 1.0 / F
N_LOC = N // N_CORES        # 2048 queries per core
N_TILES = N_LOC // 128      # 16 i-tiles of 128 queries
K_AUG = F + 4               # 68 contraction rows (x, jt hi/lo, c hi/lo)
W = 2048                    # j-window: 4 PSUM banks
NW = M // W                 # 4 windows per j sweep
MM_N = 512                  # matmul moving free dim (1 PSUM bank)

# Schraudolph constants: round(A*e + B) as int16 == bf16 bits of ~exp(e)
SCH_A = 128.0 / float(np.log(2.0))
SCH_C = 7.5

# Per-tile count of DVE windows (0..2); sum is the DVE share.
# n_dve=2 tiles run [DVE, ACT, DVE, ACT]: no same-engine adjacent
# windows, so the 2-slot PSUM round-robin never exposes matmul latency.
DVE_PER_TILE = [2] * N_TILES  # sum 32

# Leading slice of each DVE window's N piece handled by ACT (exact exp)
# to balance the three engines.  Must be even; < 1002.
AS = 512

BF16 = mybir.dt.bfloat16
I16 = mybir.dt.int16
F32 = mybir.dt.float32
bf16 = ml_dtypes.bfloat16

_compiled_cache = {}


def _build(ranges):
    """ranges: tuple of (lo, hi, is_pos) sign ranges covering [0, M)."""
    nc = bacc.Bacc(
        "TRN2",
        target_bir_lowering=False,
        debug=False,
        enable_asserts=False,
        num_devices=N_CORES,
    )
    sch_b = 16256.0 - SCH_C

    def pieces_of(w):
        lo, hi = w * W, (w + 1) * W
        out = []
        for rlo, rhi, pos in ranges:
            plo, phi = max(lo, rlo), min(hi, rhi)
            if plo < phi:
                out.append((plo, phi, pos))
        return out

    # Window order per tile interleaves the DVE and ACT windows so both
    # engines run concurrently on the two in-flight PSUM tiles.
    def tile_schedule(n_dve):
        if n_dve == 0:
            return [(2, "A"), (0, "A"), (1, "A"), (3, "A")]
        if n_dve == 1:
            return [(2, "D"), (1, "A"), (0, "A"), (3, "A")]
        return [(2, "D"), (0, "A"), (1, "D"), (3, "A")]

    n_pos = 8
    n_neg = 8

    with tile.TileContext(nc) as tc:
        with (
            tc.tile_pool(name="const", bufs=1) as cpool,
            tc.tile_pool(name="acc", bufs=3) as apool,
            tc.tile_pool(name="stg", bufs=3) as spool,
            tc.tile_pool(name="psum", bufs=2, space="PSUM") as ppool,
        ):
            xaugT_d = nc.dram_tensor(
                "xaugT", [K_AUG, N_LOC], BF16, kind="ExternalInput"
            )
            saug_d = nc.dram_tensor("saug", [K_AUG, M], BF16, kind="ExternalInput")
            out_d = nc.dram_tensor("out", [128, N_TILES], F32, kind="ExternalOutput")

            # Dummy exp() on a zeroed tile: first in the ACT engine's
            # program, so the exp table load overlaps the input DMAs.
            warm_act = cpool.tile([128, 1], F32)
            nc.gpsimd.memset(warm_act[:], 0.0)
            nc.scalar.activation(
                warm_act[:], warm_act[:], mybir.ActivationFunctionType.Exp
            )

            saug_sb = cpool.tile([K_AUG, M], BF16)
            nc.sync.dma_start(saug_sb[:, 2 * W : 3 * W], saug_d.ap()[:, 2 * W : 3 * W])
            xaugT_sb = cpool.tile([K_AUG, N_LOC], BF16)
            nc.sync.dma_start(xaugT_sb[:, 0:128], xaugT_d.ap()[:, 0:128])
            for w in (0, 1, 3):
                nc.sync.dma_start(
                    saug_sb[:, w * W : (w + 1) * W],
                    saug_d.ap()[:, w * W : (w + 1) * W],
                )
            nc.sync.dma_start(xaugT_sb[:, 128:], xaugT_d.ap()[:, 128:])

            outT_sb = cpool.tile([128, N_TILES], F32)
            dvout = cpool.tile([128, W], BF16)

            for t in range(N_TILES):
                accP = apool.tile([128, n_pos], F32, tag="accP")
                accN = apool.tile([128, n_neg], F32, tag="accN")
                iP = iN = 0

                def acc_col(pos):
                    nonlocal iP, iN
                    if pos:
                        col = accP[:, iP : iP + 1]
                        iP += 1
                    else:
                        col = accN[:, iN : iN + 1]
                        iN += 1
                    return col

                for w, eng in tile_schedule(DVE_PER_TILE[t]):
                    ps_tile = ppool.tile([128, W], F32, tag="E")
                    for c in range(W // MM_N):
                        nc.tensor.matmul(
                            ps_tile[:, c * MM_N : (c + 1) * MM_N],
                            xaugT_sb[:, t * 128 : (t + 1) * 128],
                            saug_sb[:, w * W + c * MM_N : w * W + (c + 1) * MM_N],
                            start=True,
                            stop=True,
                        )
                    if eng == "D":
                        pieces = pieces_of(w)
                        assert len(pieces) == 2 and pieces[0][2] != pieces[1][2]
                        pp = next(p for p in pieces if p[2])
                        pn = next(p for p in pieces if not p[2])
                        o = w * W
                        assert pn[0] == o and pn[1] == pp[0]
                        # ACT helps: exact exp+accum on the leading AS of
                        # the N piece (single sign, one accumulator read).
                        nc.scalar.activation(
                            ps_tile[:, 0:AS],
                            ps_tile[:, 0:AS],
                            mybir.ActivationFunctionType.Exp,
                            accum_out=acc_col(False),
                        )
                        # Schraudolph exp of the rest into int16 staging.
                        stg = spool.tile([128, W], I16, tag="stg")
                        nc.vector.tensor_scalar(
                            stg[:, AS:W],
                            ps_tile[:, AS:W],
                            SCH_A,
                            sch_b,
                            mybir.AluOpType.mult,
                            mybir.AluOpType.add,
                        )
                        stg_bf = stg[:].bitcast(BF16)
                        ln = pn[1] - pn[0] - AS   # N elements on DVE
                        lp = pp[1] - pp[0]
                        L = min(ln, lp)
                        fold = spool.tile([128, W // 2], BF16, tag="fold")
                        nc.vector.tensor_sub(
                            fold[:, 0:L],
                            stg_bf[:, pp[0] - o : pp[0] - o + L],
                            stg_bf[:, AS : AS + L],
                        )
                        # Second fold level, then reduce the quarter.
                        L2 = L // 2
                        fold2 = spool.tile([128, W // 4], BF16, tag="fold2")
                        nc.vector.tensor_add(
                            fold2[:, 0:L2], fold[:, 0:L2], fold[:, L2 : 2 * L2]
                        )
                        nc.vector.tensor_scalar(
                            dvout[:, 0:L2],
                            fold2[:, 0:L2],
                            1.0,
                            0.0,
                            mybir.AluOpType.mult,
                            mybir.AluOpType.add,
                            accum_out=acc_col(True),
                        )
                        if L % 2:
                            nc.vector.tensor_scalar(
                                dvout[:, W // 2 : W // 2 + 1],
                                fold[:, L - 1 : L],
                                1.0,
                                0.0,
                                mybir.AluOpType.mult,
                                mybir.AluOpType.add,
                                accum_out=acc_col(True),
                            )
                        if lp > L:
                            nc.vector.tensor_scalar(
                                dvout[:, W // 2 : W // 2 + (lp - L)],
                                stg_bf[:, pp[0] - o + L : pp[1] - o],
                                1.0,
                                0.0,
                                mybir.AluOpType.mult,
                                mybir.AluOpType.add,
                                accum_out=acc_col(True),
                            )
                        elif ln > L:
                            nc.vector.tensor_scalar(
                                dvout[:, W // 2 : W // 2 + (ln - L)],
                                stg_bf[:, AS + L : pn[1] - o],
                                1.0,
                                0.0,
                                mybir.AluOpType.mult,
                                mybir.AluOpType.add,
                                accum_out=acc_col(False),
                            )
                    else:
                        for lo, hi, pos in pieces_of(w):
                            nc.scalar.activation(
                                ps_tile[:, lo - w * W : hi - w * W],
                                ps_tile[:, lo - w * W : hi - w * W],
                                mybir.ActivationFunctionType.Exp,
                                accum_out=acc_col(pos),
                            )
                sumP = apool.tile([128, 1], F32, tag="sumP")
                nc.vector.reduce_sum(sumP[:], accP[:, :iP], axis=mybir.AxisListType.X)
                sumN = apool.tile([128, 1], F32, tag="sumN")
                nc.vector.reduce_sum(sumN[:], accN[:, :iN], axis=mybir.AxisListType.X)
                nc.vector.tensor_sub(outT_sb[:, t : t + 1], sumP[:], sumN[:])

            nc.sync.dma_start(out_d.ap()[:], outT_sb[:])

    nc.compile()
    return nc


def _prepare(x, supports, alphas):
    x = np.asarray(x, dtype=np.float32)
    supports = np.asarray(supports, dtype=np.float32)
    alphas = np.asarray(alphas, dtype=np.float32)

    a64 = alphas.astype(np.float64)
    s64 = supports.astype(np.float64)

    pos = a64 > 0
    iP = np.nonzero(pos)[0]
    iN = np.nonzero(~pos)[0]
    Pd = iP[np.argsort(-np.abs(a64[iP]))]  # descending |alpha|
    Nd = iN[np.argsort(-np.abs(a64[iN]))]

    # Window layout (even piece boundaries for DVE 2x alignment):
    #   w0 = P big 2048
    #   w1 = [N mid n1n | P mid n1p]
    #   w2 = [N small 1024 | P small 1024]
    #   w3 = [N big 2048-len(dead) | dead]
    # Any odd-count leftovers are parked in `dead` with jt=-50 (zero
    # weight), keeping all live piece boundaries even.
    nP, nN = len(Pd), len(Nd)
    # Park the globally smallest positives in `dead` (weight zero) until
    # the w1 P-piece count is even and slot totals work out.
    n_dead = (nP - 2048 - 1024) % 2
    n1p = nP - n_dead - 1024 - 2048
    n1n = 2048 - n1p
    assert n1n <= nN - 1024, (n1n, nN)
    w0 = Pd[0:2048]
    w1P = Pd[2048 : 2048 + n1p]
    w2P = Pd[2048 + n1p : nP - n_dead]
    dead = Pd[nP - n_dead :]
    assert len(w2P) == 1024
    w2N = Nd[nN - 1024 :]
    w1N = Nd[nN - 1024 - n1n : nN - 1024]
    # w3 region holds the big negatives, then dead.
    w3_full = np.concatenate([Nd[0 : nN - 1024 - n1n], dead])
    assert len(w3_full) == 2048, len(w3_full)

    blocks = [
        (w0, True),
        (w1N, False),
        (w1P, True),
        (w2N, False),
        (w2P, True),
        (w3_full, False),
    ]
    perm = np.concatenate([b for b, _ in blocks])
    assert len(perm) == M
    ranges = []
    o = 0
    for blk, sgn in blocks:
        ranges.append((o, o + len(blk), sgn))
        o += len(blk)
    ranges = tuple(ranges)
    n_dead = len(dead)

    jterm = -GAMMA * (s64 * s64).sum(axis=1) + np.log(
        np.maximum(np.abs(a64), 1e-300)
    )
    jt = jterm[perm]
    if n_dead:
        jt[M - n_dead :] = -50.0  # weight exp(-50) = 0
    jt_hi = jt.astype(bf16)
    jt_lo = (jt - jt_hi.astype(np.float64)).astype(bf16)

    saug = np.ones((K_AUG, M), dtype=bf16)
    saug[:F] = supports[perm].T.astype(bf16)
    saug[F] = jt_hi
    saug[F + 1] = jt_lo

    cterm = -GAMMA * (x.astype(np.float64) ** 2).sum(axis=1)
    c_hi = cterm.astype(bf16)
    c_lo = (cterm - c_hi.astype(np.float64)).astype(bf16)

    xaugT = np.ones((K_AUG, N), dtype=bf16)
    xaugT[:F] = (x.T / 32.0).astype(bf16)
    xaugT[F + 2] = c_hi
    xaugT[F + 3] = c_lo

    in_maps = []
    for c in range(N_CORES):
        sl = slice(c * N_LOC, (c + 1) * N_LOC)
        in_maps.append(
            {
                "xaugT": np.ascontiguousarray(xaugT[:, sl]),
                "saug": saug,
            }
        )
    return ranges, in_maps


def _run(x, supports, alphas, trace=False, **run_kwargs):
    ranges, in_maps = _prepare(x, supports, alphas)
    key = (ranges, tuple(DVE_PER_TILE), SCH_C)
    if key not in _compiled_cache:
        _compiled_cache[key] = _build(ranges)
    nc = _compiled_cache[key]
    res = run_bass_kernel_spmd(
        nc, in_maps, core_ids=list(range(N_CORES)), trace=trace, **run_kwargs
    )
    outs = [r["out"].T.reshape(-1) for r in res.results]
    return np.concatenate(outs).astype(np.float32), res


def kernel(x, supports, alphas):
    out, _ = _run(x, supports, alphas, trace=False)
    return out


# revision 19
# speedup vs baseline: 1.1798x; 1.1798x over previous
"""RBF-kernel SVM decision function on 8 TRN2 NeuronCores.

out[i] = sum_j alphas[j] * exp(-GAMMA * ||x[i] - supports[j]||^2)

Strategy (data-parallel over x rows, supports/alphas replicated):
  exponent e_ij = -g|x_i|^2 + (2g x_i . s_j) + (ln|a_j| - g|s_j|^2)
    - 2g x_i.s_j  : bf16 matmul, x-side scaled by 1/32 (exact pow2), s-side raw
    - j-term      : folded into the matmul as 2 extra contraction rows (hi/lo
                    bf16 split for ~fp24 accuracy), x-side rows = 1.0
    - i-term      : fp32 per-partition bias of the ACTIVATE(Exp)
  out_i = sum_{j: a_j>0} exp(e_ij) - sum_{j: a_j<0} exp(e_ij)
    - supports host-sorted so positive-alpha group comes first

Two reduction schemes:
  dve_accum (default, hybrid): per i-tile, one j-window is reduced by
    ACTIVATE(accum_out=...) in place on PSUM (one ACTIVATION_READ_ACCUMULATOR);
    the other three are written as fp16 to SBUF staging and reduced by the
    otherwise-idle DVE via tensor_scalar(accum_out=...) (1x-rate
    TENSOR_SCALAR_CACHE_REDUCE, ~1.75us/window, hidden under ScalarE). The
    sign-boundary window goes to the DVE side where an arbitrary split point
    just costs one extra op. ScalarE stays at its ~2us/window floor.
  act_accum (fallback, BASS_ACT_ACCUM=1): every window reduced by
    ACTIVATE(accum_out=...); costs an accumulator read per window plus a
    split ACTIVATE in the sign-boundary window (~7% slower overall).
"""

import os
import sys

for p in ("/opt/trn_rl_repo",):
    if p not in sys.path:
        sys.path.insert(0, p)

import numpy as np
import ml_dtypes

import concourse.bass as bass
import concourse.tile as tile
from concourse import bacc, mybir
from concourse.bass_utils import run_bass_kernel_spmd

N_CORES = 8
N = 16384
M = 8192
F = 64
GAMMA = 1.0 / F
N_LOC = N // N_CORES        # 2048 queries per core
N_TILES = N_LOC // 128      # 16 i-tiles of 128 queries
K_AUG = F + 2               # 66 contraction rows
W = 2048                    # j-window: 4 PSUM banks
NW = M // W                 # 4 windows per j sweep
MM_N = 512                  # matmul moving free dim (1 PSUM bank)
M_PAD = M + 256             # fp16 staging width (zero tail pad, mult of 4)

BF16 = mybir.dt.bfloat16
FP16 = mybir.dt.float16
F32 = mybir.dt.float32
bf16 = ml_dtypes.bfloat16

_compiled_cache = {}


def _build_common(nc, tc, cpool):
    """Input DRAM tensors, table-load warmer, and input DMAs (sync-ordered so
    the first window's operands land first)."""
    xaugT_d = nc.dram_tensor("xaugT", [K_AUG, N_LOC], BF16, kind="ExternalInput")
    saug_d = nc.dram_tensor("saug", [K_AUG, M], BF16, kind="ExternalInput")
    cbias_d = nc.dram_tensor("cbias", [128, N_TILES], F32, kind="ExternalInput")
    out_d = nc.dram_tensor("out", [128, N_TILES], F32, kind="ExternalOutput")

    # Dummy exp() on a zeroed tile: first in the ACT engine's program, so the
    # ~2.7us exp table load overlaps the input DMAs instead of stalling the
    # first real ACTIVATE.
    warm_act = cpool.tile([128, 1], F32)
    nc.gpsimd.memset(warm_act[:], 0.0)
    nc.scalar.activation(warm_act[:], warm_act[:], mybir.ActivationFunctionType.Exp)

    saug_sb = cpool.tile([K_AUG, M], BF16)
    nc.sync.dma_start(saug_sb[:, 0:W], saug_d.ap()[:, 0:W])
    xaugT_sb = cpool.tile([K_AUG, N_LOC], BF16)
    nc.sync.dma_start(xaugT_sb[:, 0:128], xaugT_d.ap()[:, 0:128])
    cbias_sb = cpool.tile([128, N_TILES], F32)
    nc.sync.dma_start(cbias_sb[:], cbias_d.ap()[:])
    for w in range(1, NW):
        nc.sync.dma_start(
            saug_sb[:, w * W : (w + 1) * W],
            saug_d.ap()[:, w * W : (w + 1) * W],
        )
    nc.sync.dma_start(xaugT_sb[:, 128:], xaugT_d.ap()[:, 128:])
    return xaugT_sb, saug_sb, cbias_sb, out_d


def _mm_windows(nc, t, ps_tile, w, xaugT_sb, saug_sb):
    for c in range(W // MM_N):
        nc.tensor.matmul(
            ps_tile[:, c * MM_N : (c + 1) * MM_N],
            xaugT_sb[:, t * 128 : (t + 1) * 128],
            saug_sb[:, w * W + c * MM_N : w * W + (c + 1) * MM_N],
            start=True,
            stop=True,
        )


def _build_dve_accum(b):
    """Hybrid reduction: one window per i-tile uses ACTIVATE(accum_out=...)
    (in-place on PSUM, one accumulator read); the other three are written as
    fp16 to SBUF staging and reduced by the otherwise-idle DVE with 1x
    TENSOR_SCALAR_CACHE_REDUCE ops. The sign-boundary window goes to the DVE,
    where an arbitrary split point costs only one extra small op."""
    nc = bacc.Bacc(
        "TRN2",
        target_bir_lowering=False,
        debug=False,
        enable_asserts=False,
        num_devices=N_CORES,
    )
    w_mix = b // W  # window containing the P/N boundary (b % W may be 0)
    act_w = 0 if w_mix != 0 else 1  # the one ACT-accum window, never mixed
    dve_ws = [w for w in range(NW) if w != act_w]

    def pieces_of(w):
        lo, hi = w * W, (w + 1) * W
        if b <= lo:
            return [(lo, hi, False)]
        if b >= hi:
            return [(lo, hi, True)]
        return [(lo, b, True), (b, hi, False)]

    n_pos = sum(1 for w in range(NW) for p in pieces_of(w) if p[2])
    n_neg = sum(1 for w in range(NW) for p in pieces_of(w) if not p[2])

    with tile.TileContext(nc) as tc:
        with (
            tc.tile_pool(name="const", bufs=1) as cpool,
            tc.tile_pool(name="acc", bufs=3) as apool,
            tc.tile_pool(name="stg", bufs=3) as spool,
            tc.tile_pool(name="psum", bufs=2, space="PSUM") as ppool,
        ):
            xaugT_sb, saug_sb, cbias_sb, out_d = _build_common(nc, tc, cpool)
            outT_sb = cpool.tile([128, N_TILES], F32)
            dvout = cpool.tile([128, M], FP16)

            for t in range(N_TILES):
                accP = apool.tile([128, max(n_pos, 1)], F32, tag="accP")
                accN = apool.tile([128, max(n_neg, 1)], F32, tag="accN")
                iP = iN = 0

                def acc_col(pos):
                    nonlocal iP, iN
                    if pos:
                        col = accP[:, iP : iP + 1]
                        iP += 1
                    else:
                        col = accN[:, iN : iN + 1]
                        iN += 1
                    return col

                # Last i-tile: ACT-accum everything (split at the sign
                # boundary) so no DVE reduce chain trails the final ACTIVATE.
                # Even i-tiles: all four windows reduced on the DVE (no
                # accumulator read on ScalarE); odd i-tiles keep one ACT-accum
                # window so the DVE stays below the ScalarE pace.
                last = t == N_TILES - 1
                if last:
                    act_set = set(range(NW))
                elif t % 2 == 0:
                    act_set = set()
                else:
                    act_set = {act_w}
                stg = spool.tile([128, M], FP16, tag="stg")
                for w in range(NW):
                    ps_tile = ppool.tile([128, W], F32, tag="E")
                    _mm_windows(nc, t, ps_tile, w, xaugT_sb, saug_sb)
                    if w in act_set:
                        for lo, hi, pos in pieces_of(w):
                            nc.scalar.activation(
                                ps_tile[:, lo - w * W : hi - w * W],
                                ps_tile[:, lo - w * W : hi - w * W],
                                mybir.ActivationFunctionType.Exp,
                                bias=cbias_sb[:, t : t + 1],
                                accum_out=acc_col(pos),
                            )
                    else:
                        nc.scalar.activation(
                            stg[:, w * W : (w + 1) * W],
                            ps_tile[:],
                            mybir.ActivationFunctionType.Exp,
                            bias=cbias_sb[:, t : t + 1],
                        )
                for w in range(NW):
                    if w in act_set:
                        continue
                    for lo, hi, pos in pieces_of(w):
                        nc.vector.tensor_scalar(
                            dvout[:, lo:hi],
                            stg[:, lo:hi],
                            1.0,
                            0.0,
                            mybir.AluOpType.mult,
                            mybir.AluOpType.add,
                            accum_out=acc_col(pos),
                        )
                sumP = apool.tile([128, 1], F32, tag="sumP")
                nc.vector.reduce_sum(sumP[:], accP[:, :iP], axis=mybir.AxisListType.X)
                sumN = apool.tile([128, 1], F32, tag="sumN")
                nc.vector.reduce_sum(sumN[:], accN[:, :iN], axis=mybir.AxisListType.X)
                nc.vector.tensor_sub(outT_sb[:, t : t + 1], sumP[:], sumN[:])

            nc.sync.dma_start(out_d.ap()[:], outT_sb[:])

    nc.compile()
    return nc


def _build_act_accum(b):
    nc = bacc.Bacc(
        "TRN2",
        target_bir_lowering=False,
        debug=False,
        enable_asserts=False,
        num_devices=N_CORES,
    )
    n_pos = sum(1 for w in range(NW) if w * W < b)
    n_neg = sum(1 for w in range(NW) if (w + 1) * W > b)

    with tile.TileContext(nc) as tc:
        with (
            tc.tile_pool(name="const", bufs=1) as cpool,
            tc.tile_pool(name="acc", bufs=3) as apool,
            tc.tile_pool(name="psum", bufs=2, space="PSUM") as ppool,
        ):
            xaugT_sb, saug_sb, cbias_sb, out_d = _build_common(nc, tc, cpool)
            outT_sb = cpool.tile([128, N_TILES], F32)

            for t in range(N_TILES):
                accP = apool.tile([128, max(n_pos, 1)], F32, tag="accP")
                accN = apool.tile([128, max(n_neg, 1)], F32, tag="accN")
                iP = iN = 0
                for w in range(NW):
                    ps_tile = ppool.tile([128, W], F32, tag="E")
                    _mm_windows(nc, t, ps_tile, w, xaugT_sb, saug_sb)
                    lo, hi = w * W, (w + 1) * W
                    if b <= lo:
                        pieces = [(lo, hi, False)]
                    elif b >= hi:
                        pieces = [(lo, hi, True)]
                    else:
                        pieces = [(lo, b, True), (b, hi, False)]
                    for plo, phi, pos in pieces:
                        if pos:
                            acc_col = accP[:, iP : iP + 1]
                            iP += 1
                        else:
                            acc_col = accN[:, iN : iN + 1]
                            iN += 1
                        nc.scalar.activation(
                            ps_tile[:, plo - lo : phi - lo],
                            ps_tile[:, plo - lo : phi - lo],
                            mybir.ActivationFunctionType.Exp,
                            bias=cbias_sb[:, t : t + 1],
                            accum_out=acc_col,
                        )
                sumP = apool.tile([128, 1], F32, tag="sumP")
                nc.vector.reduce_sum(sumP[:], accP[:, :iP], axis=mybir.AxisListType.X)
                sumN = apool.tile([128, 1], F32, tag="sumN")
                nc.vector.reduce_sum(sumN[:], accN[:, :iN], axis=mybir.AxisListType.X)
                nc.vector.tensor_sub(outT_sb[:, t : t + 1], sumP[:], sumN[:])

            nc.sync.dma_start(out_d.ap()[:], outT_sb[:])

    nc.compile()
    return nc


def _build(b):
    if os.environ.get("BASS_ACT_ACCUM"):
        return _build_act_accum(b)
    return _build_dve_accum(b)


def _prepare(x, supports, alphas):
    x = np.asarray(x, dtype=np.float32)
    supports = np.asarray(supports, dtype=np.float32)
    alphas = np.asarray(alphas, dtype=np.float32)

    a64 = alphas.astype(np.float64)
    s64 = supports.astype(np.float64)
    jterm = -GAMMA * (s64 * s64).sum(axis=1) + np.log(
        np.maximum(np.abs(a64), 1e-300)
    )

    pos = a64 > 0
    perm = np.concatenate([np.nonzero(pos)[0], np.nonzero(~pos)[0]])
    b = int(pos.sum())

    jt = jterm[perm]
    hi = jt.astype(bf16)
    lo = (jt - hi.astype(np.float64)).astype(bf16)

    saug = np.empty((K_AUG, M), dtype=bf16)
    saug[:F] = supports[perm].T.astype(bf16)
    saug[F] = hi
    saug[F + 1] = lo

    xaugT = np.ones((K_AUG, N), dtype=bf16)
    xaugT[:F] = (x.T / 32.0).astype(bf16)

    cbias = (-GAMMA * (x.astype(np.float64) ** 2).sum(axis=1)).astype(np.float32)

    in_maps = []
    for c in range(N_CORES):
        sl = slice(c * N_LOC, (c + 1) * N_LOC)
        in_maps.append(
            {
                "xaugT": np.ascontiguousarray(xaugT[:, sl]),
                "saug": saug,
                "cbias": np.ascontiguousarray(
                    cbias[sl].reshape(N_TILES, 128).T
                ),
            }
        )
    return b, in_maps


def _run(x, supports, alphas, trace=False, **run_kwargs):
    b, in_maps = _prepare(x, supports, alphas)
    key = (b, bool(os.environ.get("BASS_ACT_ACCUM")))
    if key not in _compiled_cache:
        _compiled_cache[key] = _build(b)
    nc = _compiled_cache[key]
    res = run_bass_kernel_spmd(
        nc, in_maps, core_ids=list(range(N_CORES)), trace=trace, **run_kwargs
    )
    outs = [r["out"].T.reshape(-1) for r in res.results]
    return np.concatenate(outs).astype(np.float32), res


def kernel(x, supports, alphas):
    out, _ = _run(x, supports, alphas, trace=False)
    return out

